# revision 1
# baseline (speedup 1.0000x reference)
"""MetapathAggrLayer Trainium2 kernel — v2 (custom DVE ops).

Per node n: e_m = leakyrelu(x[m,n,:].a), w = softmax(e), out = sum_m w_m x[m,n,:].
Data-parallel over N across 8 NeuronCores; nodes-on-partitions layout.

v2: scores via a fused multiply+prefix-scan custom DVE op (segment sums
recovered as prefix differences at chunk boundaries), weighted sum via a
dual-MAC custom op (x0*w0 + x1*w1 per instruction), pair-combine adds on
GpSimd to offload the Vector engine.
"""

import sys

sys.path.insert(0, "/opt/trn_rl_repo")

import numpy as np

import concourse.bacc as bacc
import concourse.mybir as mybir
from concourse import bass_utils, dve_ops
from concourse.dve_spec import Spec, Src0, Src1, C0, C1, scan, maxx, AluOp, lower, _has_src1
from concourse.dve_uop import DveOpSpec
from concourse.tile import TileContext

ALPHA = 0.2
NMETA = 4
F = 64
N_FULL = 1_000_000
N_CORES = 8
T = 16                     # chunks (nodes per partition) per macro-tile
NODES_PER_MACRO = 128 * T  # 2048
MACROS_PER_CORE = 62
NC_NODES = MACROS_PER_CORE * NODES_PER_MACRO  # 126_976
N_PAD = N_CORES * NC_NODES                    # 1_015_808

MAC_ADD_ENGINE = "gpsimd"  # "gpsimd" | "vector"

_CACHE = {}


def _register_op(name, spec, subdim=False):
    if name in dve_ops._SUB_OPCODE_FOR_NAME:
        return next(o for o in dve_ops.OPS if o.name == name)
    row = dve_ops._CUSTOM_DVE_ROW_BASE + len(dve_ops.OPS)
    assert row < 0x20
    shas = {}
    for ver in ("v3", "v4"):
        s = DveOpSpec(name=name, opcode=row, uops=lower(spec, ver=ver),
                      rd1_en=_has_src1(spec))
        shas[ver] = s.sha(ver)
    op = dve_ops.DveOp(name, spec, subdim, shas)
    dve_ops.OPS.append(op)
    dve_ops.CUSTOM_DVE_SPECS[name] = spec
    dve_ops._SUB_OPCODE_FOR_NAME[name] = row
    return op


def _get_ops():
    scan_mul = _register_op(
        "MPA_SCAN_MUL",
        Spec(
            body=scan(AluOp.ADD, Src0 * Src1),
            reference=lambda in0, in1, s0, s1: np.cumsum(
                (in0.astype(np.float32) * in1.astype(np.float32)), axis=-1
            ),
        ),
    )
    ext_lrelu = _register_op(
        "MPA_EXT_LRELU",
        Spec(
            body=(lambda d: maxx(d, d * C0))(Src0 - Src1),
            reference=lambda in0, in1, s0, s1: np.maximum(in0 - in1, (in0 - in1) * s0),
        ),
    )
    dual_mac = _register_op(
        "MPA_DUAL_MAC",
        Spec(
            body=Src0 * C0 + Src1 * C1,
            reference=lambda in0, in1, s0, s1: in0 * s0 + in1 * s1,
        ),
    )
    return scan_mul, dual_mac, ext_lrelu


def _build_kernel():
    scan_mul, dual_mac, ext_lrelu = _get_ops()

    nc = bacc.Bacc("TRN2", target_bir_lowering=False, debug=False)
    dt = mybir.dt.float32

    x_in = nc.dram_tensor("input", (NMETA, NC_NODES, F), dt, kind="ExternalInput").ap()
    a_rep_in = nc.dram_tensor("a_rep", (128, T * F), dt, kind="ExternalInput").ap()
    out = nc.dram_tensor("out", (NC_NODES, F), dt, kind="ExternalOutput").ap()

    mult = mybir.AluOpType.mult
    add = mybir.AluOpType.add
    subtract = mybir.AluOpType.subtract
    op_max = mybir.AluOpType.max

    with TileContext(nc) as tc:
        with tc.tile_pool(name="const", bufs=1) as cpool, \
             tc.tile_pool(name="sbuf", bufs=3) as pool, \
             tc.tile_pool(name="scratch", bufs=2) as scpool, \
             tc.tile_pool(name="small", bufs=4) as spool:
            a_rep = cpool.tile([128, T * F], dt)
            nc.sync.dma_start(out=a_rep[:, :], in_=a_rep_in)

            for i in range(MACROS_PER_CORE):
                lo = i * NODES_PER_MACRO
                hi = lo + NODES_PER_MACRO

                xt = []
                for m in range(NMETA):
                    src = x_in[m, lo:hi, :].rearrange("(p t) f -> p (t f)", p=128)
                    xm = pool.tile([128, T * F], dt, tag=f"x{m}")
                    nc.sync.dma_start(out=xm[:, :], in_=src)
                    xt.append(xm)

                # ---- scores: prefix scan of x*a, segment sums by differencing
                e = spool.tile([128, NMETA * T], dt, tag="e")
                for m in range(NMETA):
                    pm = scpool.tile([128, T * F + 1], dt, tag=f"P{m}")
                    nc.gpsimd.memset(pm[:, 0:1], 0.0)
                    nc.vector._custom_dve(
                        scan_mul, out=pm[:, 1:T * F + 1],
                        in0=xt[m][:, :], in1=a_rep[:, :],
                    )
                    p_hi = pm[:, 1:T * F + 1].rearrange(
                        "p (t f) -> p t f", f=F)[:, :, F - 1:F]
                    p_lo = pm[:, 0:T * F].rearrange(
                        "p (t f) -> p t f", f=F)[:, :, 0:1]
                    nc.vector.tensor_tensor(
                        out=e[:, m * T:(m + 1) * T], in0=p_hi, in1=p_lo, op=subtract
                    )

                # ---- leakyrelu on DVE, exp on ScalarE
                u = spool.tile([128, NMETA * T], dt, tag="u")
                et = spool.tile([128, NMETA * T], dt, tag="et")
                nc.vector.tensor_scalar_mul(et[:, :], e[:, :], ALPHA)
                nc.vector.tensor_tensor(out=et[:, :], in0=e[:, :], in1=et[:, :], op=op_max)
                nc.scalar.activation(u[:, :], et[:, :], mybir.ActivationFunctionType.Exp)

                # ---- s = sum_m u_m ; r = 1/s ; w_m = u_m * r
                s01 = spool.tile([128, T], dt, tag="s01")
                s23 = spool.tile([128, T], dt, tag="s23")
                s = spool.tile([128, T], dt, tag="s")
                nc.vector.tensor_tensor(out=s01[:, :], in0=u[:, 0:T], in1=u[:, T:2 * T], op=add)
                nc.vector.tensor_tensor(out=s23[:, :], in0=u[:, 2 * T:3 * T], in1=u[:, 3 * T:4 * T], op=add)
                nc.vector.tensor_tensor(out=s[:, :], in0=s01[:, :], in1=s23[:, :], op=add)
                r = spool.tile([128, T], dt, tag="r")
                nc.vector.reciprocal(r[:, :], s[:, :])
                w = spool.tile([128, NMETA * T], dt, tag="w")
                r_bc = r[:, :].rearrange("p (o t) -> p o t", o=1).broadcast_to(
                    [128, NMETA, T])
                u_3d = u[:, :].rearrange("p (m t) -> p m t", m=NMETA)
                w_3d = w[:, :].rearrange("p (m t) -> p m t", m=NMETA)
                nc.vector.tensor_tensor(out=w_3d, in0=u_3d, in1=r_bc, op=mult)

                # ---- weighted sum: pair (0,1) dual-MAC on DVE; metapaths 2,3
                # scaled on ScalarE (activation Copy, per-partition scale);
                # combined with two full-width GpSimd adds.
                acc = scpool.tile([128, T * F], dt, tag="acc")
                acc1 = scpool.tile([128, T * F], dt, tag="acc1")
                t01 = scpool.tile([128, T * F], dt, tag="t01")
                t2 = scpool.tile([128, T * F], dt, tag="t2")
                t3 = scpool.tile([128, T * F], dt, tag="t3")
                for t in range(T):
                    fs = t * F
                    nc.vector._custom_dve(
                        dual_mac, out=t01[:, fs:fs + F],
                        in0=xt[0][:, fs:fs + F], in1=xt[1][:, fs:fs + F],
                        s0=w[:, t:t + 1], s1=w[:, T + t:T + t + 1],
                    )
                    nc.scalar.mul(t2[:, fs:fs + F], xt[2][:, fs:fs + F],
                                  w[:, 2 * T + t:2 * T + t + 1])
                    nc.scalar.mul(t3[:, fs:fs + F], xt[3][:, fs:fs + F],
                                  w[:, 3 * T + t:3 * T + t + 1])
                nc.gpsimd.tensor_tensor(out=acc1[:, :], in0=t01[:, :], in1=t2[:, :], op=add)
                nc.gpsimd.tensor_tensor(out=acc[:, :], in0=acc1[:, :], in1=t3[:, :], op=add)

                dst = out[lo:hi, :].rearrange("(p t) f -> p (t f)", p=128)
                nc.sync.dma_start(out=dst, in_=acc[:, :])

    nc.compile()
    return nc


def kernel(input, a, _trace=False):
    input = np.ascontiguousarray(np.asarray(input, dtype=np.float32))
    a = np.asarray(a, dtype=np.float32).reshape(F)

    if "nc" not in _CACHE:
        _CACHE["nc"] = _build_kernel()
    nc = _CACHE["nc"]

    pad = N_PAD - input.shape[1]
    xp = np.concatenate(
        [input, np.zeros((NMETA, pad, F), np.float32)], axis=1
    ) if pad else input

    a_rep = np.tile(a[None, :], (128, T)).astype(np.float32)

    in_maps = []
    for c in range(N_CORES):
        sl = xp[:, c * NC_NODES:(c + 1) * NC_NODES, :]
        in_maps.append({"input": np.ascontiguousarray(sl), "a_rep": a_rep})

    res = bass_utils.run_bass_kernel_spmd(
        nc, in_maps, core_ids=list(range(N_CORES)), trace=_trace
    )
    outs = [res.results[c]["out"] for c in range(N_CORES)]
    full = np.concatenate(outs, axis=0)[:N_FULL]
    if _trace:
        return full, res
    return full



# revision 5
# speedup vs baseline: 1.3364x; 1.3364x over previous
"""MetapathAggrLayer Trainium2 kernel — v3 (fp16 + TensorE metapath-sum).

Per node n: e_m = leakyrelu(x[m,n,:].a), w = softmax(e), out = sum_m w_m x[m,n,:].
Data-parallel over N across 8 NeuronCores; nodes-on-partitions layout.

v3 design (vs v2's custom-DVE ops):
  * inputs cast to fp16 on host — halves HBM read traffic and doubles DVE
    tensor_tensor throughput (2x packed mode needs unit strides + 16-bit).
  * scores: one big fp16 multiply (x ⊙ a_rep) + one segmented reduce
    (f-innermost) per 2-macro batch instead of custom prefix scans.
  * softmax pointwise (leakyrelu, exp) on the Scalar engine's native
    activation functions.
  * weighted sum: 4 hadamards with stride-0-broadcast w (split DVE/GpSimd),
    summed over metapaths on the otherwise-idle TensorE via identity-matmul
    PSUM accumulation; PSUM drained to SBUF as fp16 by the Scalar engine.
"""

import sys

sys.path.insert(0, "/opt/trn_rl_repo")

import numpy as np

import concourse.bacc as bacc
import concourse.mybir as mybir
from concourse import bass_utils
from concourse.tile import TileContext

ALPHA = 0.2
NMETA = 4
F = 64
N_FULL = 1_000_000
N_CORES = 8
T = 32                     # node-chunks per partition per batch
NODES_PER_BATCH = 128 * T  # 4096
BATCHES_PER_CORE = 31
NC_NODES = BATCHES_PER_CORE * NODES_PER_BATCH  # 126_976
N_PAD = N_CORES * NC_NODES                     # 1_015_808
SEG = NMETA * T            # 128 score segments per partition per batch
W_CAT = T * F              # 2048: free width of one metapath tile
W_ALL = NMETA * W_CAT      # 8192: free width of the concatenated x tile

# which engine computes the hadamard y_m = w_m * x_m for each metapath
HAD_ENGINE = ("vector", "vector", "vector", "vector")

_CACHE = {}


def _build_kernel():
    nc = bacc.Bacc("TRN2", target_bir_lowering=False, debug=False)
    f16 = mybir.dt.float16
    f32 = mybir.dt.float32

    x_in = nc.dram_tensor("input", (NMETA, NC_NODES, F), f16, kind="ExternalInput").ap()
    a_rep_in = nc.dram_tensor("a_rep", (128, W_ALL), f16, kind="ExternalInput").ap()
    ident_in = nc.dram_tensor("ident", (128, 128), f16, kind="ExternalInput").ap()
    out = nc.dram_tensor("out", (NC_NODES, F), f16, kind="ExternalOutput").ap()

    mult = mybir.AluOpType.mult
    AF = mybir.ActivationFunctionType

    with TileContext(nc) as tc:
        with tc.tile_pool(name="const", bufs=1) as cpool, \
             tc.tile_pool(name="xbuf", bufs=3) as xpool, \
             tc.tile_pool(name="work", bufs=2) as wpool, \
             tc.tile_pool(name="small", bufs=2) as spool, \
             tc.tile_pool(name="psum", bufs=2, space="PSUM") as ppool:
            a_rep = cpool.tile([128, W_ALL], f16)
            ident = cpool.tile([128, 128], f16)
            alpha_c = cpool.tile([128, 1], f32)
            nc.sync.dma_start(out=a_rep[:, :], in_=a_rep_in)
            nc.sync.dma_start(out=ident[:, :], in_=ident_in)
            nc.gpsimd.memset(alpha_c[:, :], ALPHA)

            for i in range(BATCHES_PER_CORE):
                lo = i * NODES_PER_BATCH
                hi = lo + NODES_PER_BATCH

                # ---- load the 4 metapath slices into one wide tile
                xc = xpool.tile([128, W_ALL], f16, tag="xc")
                for m in range(NMETA):
                    src = x_in[m, lo:hi, :].rearrange("(p t) f -> p (t f)", p=128)
                    nc.sync.dma_start(out=xc[:, m * W_CAT:(m + 1) * W_CAT], in_=src)

                # ---- scores: e[p, m*T+t] = sum_f x*a  (big mult + seg reduce)
                prod = wpool.tile([128, W_ALL], f16, tag="prod")
                nc.vector.tensor_tensor(out=prod[:, :], in0=xc[:, :],
                                        in1=a_rep[:, :], op=mult)
                e_raw = spool.tile([128, SEG], f32, tag="e_raw")
                nc.vector.tensor_reduce(
                    out=e_raw[:, :],
                    in_=prod[:, :].rearrange("p (s f) -> p s f", f=F),
                    axis=mybir.AxisListType.X,
                    op=mybir.AluOpType.add,
                )

                # ---- softmax pieces: lrelu+exp on Scalar, sums/recip on DVE
                u = spool.tile([128, SEG], f32, tag="u")
                nc.scalar.activation(u[:, :], e_raw[:, :], AF.Prelu,
                                     alpha=alpha_c[:, :])
                nc.scalar.activation(u[:, :], u[:, :], AF.Exp)

                s = spool.tile([128, T], f32, tag="s")
                nc.vector.tensor_reduce(
                    out=s[:, :],
                    in_=u[:, :].rearrange("p (m t) -> p t m", m=NMETA),
                    axis=mybir.AxisListType.X,
                    op=mybir.AluOpType.add,
                )
                r = spool.tile([128, T], f32, tag="r")
                nc.vector.reciprocal(r[:, :], s[:, :])
                w = spool.tile([128, SEG], f16, tag="w")
                r_bc = r[:, :].rearrange("p (o t) -> p o t", o=1).broadcast_to(
                    [128, NMETA, T])
                nc.vector.tensor_tensor(
                    out=w[:, :].rearrange("p (m t) -> p m t", m=NMETA),
                    in0=u[:, :].rearrange("p (m t) -> p m t", m=NMETA),
                    in1=r_bc, op=mult)

                # ---- weighted sum: y_m = w_m (bcast over f) * x_m
                y = wpool.tile([128, W_ALL], f16, tag="y")
                for m in range(NMETA):
                    w_bc = w[:, m * T:(m + 1) * T].rearrange(
                        "p (t o) -> p t o", o=1).broadcast_to([128, T, F])
                    xm3 = xc[:, m * W_CAT:(m + 1) * W_CAT].rearrange(
                        "p (t f) -> p t f", f=F)
                    ym3 = y[:, m * W_CAT:(m + 1) * W_CAT].rearrange(
                        "p (t f) -> p t f", f=F)
                    eng = nc.vector if HAD_ENGINE[m] == "vector" else nc.gpsimd
                    eng.tensor_tensor(out=ym3, in0=xm3, in1=w_bc, op=mult)

                # ---- sum over m on TensorE: psum[:, j] += I.T @ y_m[:, j]
                acc = ppool.tile([128, W_CAT], f32, tag="acc")
                for j in range(W_CAT // 512):
                    js = j * 512
                    for m in range(NMETA):
                        nc.tensor.matmul(
                            acc[:, js:js + 512],
                            ident[:, :],
                            y[:, m * W_CAT + js:m * W_CAT + js + 512],
                            start=(m == 0),
                            stop=(m == NMETA - 1),
                        )

                # ---- drain PSUM -> SBUF (fp16) on Scalar, then store
                ot = wpool.tile([128, W_CAT], f16, tag="ot")
                nc.scalar.copy(ot[:, :], acc[:, :])
                dst = out[lo:hi, :].rearrange("(p t) f -> p (t f)", p=128)
                nc.sync.dma_start(out=dst, in_=ot[:, :])

    nc.compile()
    return nc


def kernel(input, a, _trace=False):
    a = np.asarray(a, dtype=np.float32).reshape(F)

    if "nc" not in _CACHE:
        _CACHE["nc"] = _build_kernel()
    nc = _CACHE["nc"]

    x16 = np.asarray(input).astype(np.float16)
    pad = N_PAD - x16.shape[1]
    if pad:
        x16 = np.concatenate(
            [x16, np.zeros((NMETA, pad, F), np.float16)], axis=1)

    a16 = a.astype(np.float16)
    a_rep = np.tile(a16[None, :], (128, W_ALL // F))
    ident = np.eye(128, dtype=np.float16)

    in_maps = []
    for c in range(N_CORES):
        sl = x16[:, c * NC_NODES:(c + 1) * NC_NODES, :]
        in_maps.append({
            "input": np.ascontiguousarray(sl),
            "a_rep": a_rep,
            "ident": ident,
        })

    res = bass_utils.run_bass_kernel_spmd(
        nc, in_maps, core_ids=list(range(N_CORES)), trace=_trace
    )
    outs = [res.results[c]["out"] for c in range(N_CORES)]
    full = np.concatenate(outs, axis=0)[:N_FULL].astype(np.float32)
    if _trace:
        return full, res
    return full


# revision 6
# speedup vs baseline: 1.4465x; 1.0824x over previous
"""MetapathAggrLayer Trainium2 kernel — v5 (fp16, fold-tree scores, engine-balanced).

Per node n: e_m = leakyrelu(x[m,n,:].a), w = softmax(e), out = sum_m w_m x[m,n,:].
Data-parallel over N across 8 NeuronCores; nodes-on-partitions layout.

Engine budget per 4096-node batch (measured):
  DVE:    score-mult @2x (4.9us) + fold-tree seg-sum @2x (5.5us) + softmax
          smalls (0.7us) + 1 packed hadamard (1.2us)
  GpSimd: 3 broadcast hadamards (10.8us)
  Scalar: prelu+exp (1us) + W0 expansion (2us) + PSUM drain (2us)
  TensorE: metapath sum via identity-matmul PSUM accumulation (12.6us)
  DMA:    2.5 MB (9us)
"""

import sys

sys.path.insert(0, "/opt/trn_rl_repo")

import numpy as np

import concourse.bacc as bacc
import concourse.mybir as mybir
from concourse import bass_utils
from concourse.tile import TileContext

ALPHA = 0.2
NMETA = 4
F = 64
N_FULL = 1_000_000
N_CORES = 8
T = 32                     # node-chunks per partition per batch
NODES_PER_BATCH = 128 * T  # 4096
BATCHES_PER_CORE = 31
NC_NODES = BATCHES_PER_CORE * NODES_PER_BATCH  # 126_976
N_PAD = N_CORES * NC_NODES                     # 1_015_808
SEG = NMETA * T            # 128 score segments per partition per batch
W_CAT = T * F              # 2048: free width of one metapath tile
W_ALL = NMETA * W_CAT      # 8192: free width of the concatenated x tile

_CACHE = {}


def _build_kernel():
    nc = bacc.Bacc("TRN2", target_bir_lowering=False, debug=False)
    f16 = mybir.dt.float16
    f32 = mybir.dt.float32

    x_in = nc.dram_tensor("input", (NMETA, NC_NODES, F), f16, kind="ExternalInput").ap()
    a_rep_in = nc.dram_tensor("a_rep", (128, F), f16, kind="ExternalInput").ap()
    ident_in = nc.dram_tensor("ident", (128, 128), f16, kind="ExternalInput").ap()
    out = nc.dram_tensor("out", (NC_NODES, F), f16, kind="ExternalOutput").ap()

    mult = mybir.AluOpType.mult
    add = mybir.AluOpType.add
    AF = mybir.ActivationFunctionType

    with TileContext(nc) as tc:
        with tc.tile_pool(name="const", bufs=1) as cpool, \
             tc.tile_pool(name="xbuf", bufs=3) as xpool, \
             tc.tile_pool(name="work", bufs=2) as wpool, \
             tc.tile_pool(name="small", bufs=2) as spool, \
             tc.tile_pool(name="psum", bufs=2, space="PSUM") as ppool:
            a_rep = cpool.tile([128, F], f16)
            ident = cpool.tile([128, 128], f16)
            alpha_c = cpool.tile([128, 1], f32)
            nc.sync.dma_start(out=a_rep[:, :], in_=a_rep_in)
            nc.sync.dma_start(out=ident[:, :], in_=ident_in)
            nc.gpsimd.memset(alpha_c[:, :], ALPHA)

            for i in range(BATCHES_PER_CORE):
                lo = i * NODES_PER_BATCH
                hi = lo + NODES_PER_BATCH

                # ---- load the 4 metapath slices into one wide tile
                xc = xpool.tile([128, W_ALL], f16, tag="xc")
                for m in range(NMETA):
                    src = x_in[m, lo:hi, :].rearrange("(p t) f -> p (t f)", p=128)
                    nc.sync.dma_start(out=xc[:, m * W_CAT:(m + 1) * W_CAT], in_=src)

                # ---- scores: prod = x*a (2x packed; a_rep seg-broadcast view)
                prod = wpool.tile([128, W_ALL], f16, tag="prod")
                a_bc = a_rep[:, :].rearrange("p (o f) -> p o f", o=1).broadcast_to(
                    [128, SEG, F])
                nc.vector.tensor_tensor(
                    out=prod[:, :].rearrange("p (s f) -> p s f", f=F),
                    in0=xc[:, :].rearrange("p (s f) -> p s f", f=F),
                    in1=a_bc, op=mult)

                # ---- seg-sum via fold tree (all @2x packed; halves each level)
                widths = [32, 16, 8, 4, 2, 1]
                cur = prod
                cw = F
                for lvl, hw in enumerate(widths):
                    dt_out = f32 if hw == 1 else f16
                    nxt = (spool if hw == 1 else wpool).tile(
                        [128, SEG * hw], dt_out, tag=f"fold{lvl}")
                    cin = cur[:, :].rearrange("p (s h) -> p s h", h=cw)
                    nc.vector.tensor_tensor(
                        out=nxt[:, :].rearrange("p (s h) -> p s h", h=hw),
                        in0=cin[:, :, 0:hw], in1=cin[:, :, hw:cw], op=add)
                    cur = nxt
                    cw = hw
                e_raw = cur  # [128, SEG] fp32

                # ---- softmax pieces: prelu+exp on Scalar, sums/recip on DVE
                u = spool.tile([128, SEG], f32, tag="u")
                nc.scalar.activation(u[:, :], e_raw[:, :], AF.Prelu,
                                     alpha=alpha_c[:, :])
                nc.scalar.activation(u[:, :], u[:, :], AF.Exp)

                s = spool.tile([128, T], f32, tag="s")
                nc.vector.tensor_reduce(
                    out=s[:, :],
                    in_=u[:, :].rearrange("p (m t) -> p t m", m=NMETA),
                    axis=mybir.AxisListType.X,
                    op=add,
                )
                r = spool.tile([128, T], f32, tag="r")
                nc.vector.reciprocal(r[:, :], s[:, :])
                w = spool.tile([128, SEG], f16, tag="w")
                r_bc = r[:, :].rearrange("p (o t) -> p o t", o=1).broadcast_to(
                    [128, NMETA, T])
                nc.vector.tensor_tensor(
                    out=w[:, :].rearrange("p (m t) -> p m t", m=NMETA),
                    in0=u[:, :].rearrange("p (m t) -> p m t", m=NMETA),
                    in1=r_bc, op=mult)

                # ---- weighted sum: y_m = w_m (bcast over f) * x_m
                # m=0: Scalar expands W0, DVE multiplies packed; m=1..3 GpSimd.
                y = wpool.tile([128, W_ALL], f16, tag="y")
                w0x = spool.tile([128, W_CAT], f16, tag="w0x")
                w0_bc = w[:, 0:T].rearrange("p (t o) -> p t o", o=1).broadcast_to(
                    [128, T, F])
                nc.scalar.activation(
                    w0x[:, :].rearrange("p (t f) -> p t f", f=F), w0_bc, AF.Copy)
                nc.vector.tensor_tensor(
                    out=y[:, 0:W_CAT], in0=xc[:, 0:W_CAT], in1=w0x[:, :], op=mult)
                for m in range(1, NMETA):
                    w_bc = w[:, m * T:(m + 1) * T].rearrange(
                        "p (t o) -> p t o", o=1).broadcast_to([128, T, F])
                    xm3 = xc[:, m * W_CAT:(m + 1) * W_CAT].rearrange(
                        "p (t f) -> p t f", f=F)
                    ym3 = y[:, m * W_CAT:(m + 1) * W_CAT].rearrange(
                        "p (t f) -> p t f", f=F)
                    nc.gpsimd.tensor_tensor(out=ym3, in0=xm3, in1=w_bc, op=mult)

                # ---- sum over m on TensorE: psum[:, j] += I.T @ y_m[:, j]
                acc = ppool.tile([128, W_CAT], f32, tag="acc")
                for j in range(W_CAT // 512):
                    js = j * 512
                    for m in range(NMETA):
                        nc.tensor.matmul(
                            acc[:, js:js + 512],
                            ident[:, :],
                            y[:, m * W_CAT + js:m * W_CAT + js + 512],
                            start=(m == 0),
                            stop=(m == NMETA - 1),
                        )

                # ---- drain PSUM -> SBUF (fp16) on Scalar, then store
                ot = wpool.tile([128, W_CAT], f16, tag="ot")
                nc.scalar.copy(ot[:, :], acc[:, :])
                dst = out[lo:hi, :].rearrange("(p t) f -> p (t f)", p=128)
                nc.sync.dma_start(out=dst, in_=ot[:, :])

    nc.compile()
    return nc


def kernel(input, a, _trace=False):
    a = np.asarray(a, dtype=np.float32).reshape(F)

    if "nc" not in _CACHE:
        _CACHE["nc"] = _build_kernel()
    nc = _CACHE["nc"]

    x16 = np.asarray(input).astype(np.float16)
    pad = N_PAD - x16.shape[1]
    if pad:
        x16 = np.concatenate(
            [x16, np.zeros((NMETA, pad, F), np.float16)], axis=1)

    a16 = a.astype(np.float16)
    a_rep = np.tile(a16[None, :], (128, 1))
    ident = np.eye(128, dtype=np.float16)

    in_maps = []
    for c in range(N_CORES):
        sl = x16[:, c * NC_NODES:(c + 1) * NC_NODES, :]
        in_maps.append({
            "input": np.ascontiguousarray(sl),
            "a_rep": a_rep,
            "ident": ident,
        })

    res = bass_utils.run_bass_kernel_spmd(
        nc, in_maps, core_ids=list(range(N_CORES)), trace=_trace
    )
    outs = [res.results[c]["out"] for c in range(N_CORES)]
    full = np.concatenate(outs, axis=0)[:N_FULL].astype(np.float32)
    if _trace:
        return full, res
    return full


# revision 7
# speedup vs baseline: 1.5082x; 1.0426x over previous
"""MetapathAggrLayer Trainium2 kernel — v5 (fp16, fold-tree scores, engine-balanced).

Per node n: e_m = leakyrelu(x[m,n,:].a), w = softmax(e), out = sum_m w_m x[m,n,:].
Data-parallel over N across 8 NeuronCores; nodes-on-partitions layout.

Engine budget per 4096-node batch (measured):
  DVE:    score-mult @2x (4.9us) + fold-tree seg-sum @2x (5.5us) + softmax
          smalls (0.7us) + 1 packed hadamard (1.2us)
  GpSimd: 3 broadcast hadamards (10.8us)
  Scalar: prelu+exp (1us) + W0 expansion (2us) + PSUM drain (2us)
  TensorE: metapath sum via identity-matmul PSUM accumulation (12.6us)
  DMA:    2.5 MB (9us)
"""

import sys

sys.path.insert(0, "/opt/trn_rl_repo")

import numpy as np

import concourse.bacc as bacc
import concourse.mybir as mybir
from concourse import bass_utils
from concourse.tile import TileContext

ALPHA = 0.2
NMETA = 4
F = 64
N_FULL = 1_000_000
N_CORES = 8
T = 32                     # node-chunks per partition per batch
NODES_PER_BATCH = 128 * T  # 4096
BATCHES_PER_CORE = 31
NC_NODES = BATCHES_PER_CORE * NODES_PER_BATCH  # 126_976
N_PAD = N_CORES * NC_NODES                     # 1_015_808
SEG = NMETA * T            # 128 score segments per partition per batch
W_CAT = T * F              # 2048: free width of one metapath tile
W_ALL = NMETA * W_CAT      # 8192: free width of the concatenated x tile

_CACHE = {}


def _build_kernel():
    nc = bacc.Bacc("TRN2", target_bir_lowering=False, debug=False)
    f16 = mybir.dt.float16
    f32 = mybir.dt.float32

    x_in = nc.dram_tensor("input", (NMETA, NC_NODES, F), f16, kind="ExternalInput").ap()
    a_rep_in = nc.dram_tensor("a_rep", (128, F), f16, kind="ExternalInput").ap()
    ident_in = nc.dram_tensor("ident", (128, 128), f16, kind="ExternalInput").ap()
    out = nc.dram_tensor("out", (NC_NODES, F), f16, kind="ExternalOutput").ap()

    mult = mybir.AluOpType.mult
    add = mybir.AluOpType.add
    AF = mybir.ActivationFunctionType

    with TileContext(nc) as tc:
        with tc.tile_pool(name="const", bufs=1) as cpool, \
             tc.tile_pool(name="xbuf", bufs=3) as xpool, \
             tc.tile_pool(name="work", bufs=2) as wpool, \
             tc.tile_pool(name="small", bufs=2) as spool, \
             tc.tile_pool(name="psum", bufs=2, space="PSUM") as ppool:
            a_rep = cpool.tile([128, F], f16)
            ident = cpool.tile([128, 128], f16)
            alpha_c = cpool.tile([128, 1], f32)
            nc.sync.dma_start(out=a_rep[:, :], in_=a_rep_in)
            nc.sync.dma_start(out=ident[:, :], in_=ident_in)
            nc.gpsimd.memset(alpha_c[:, :], ALPHA)

            for i in range(BATCHES_PER_CORE):
                lo = i * NODES_PER_BATCH
                hi = lo + NODES_PER_BATCH

                # ---- load the 4 metapath slices into one wide tile
                xc = xpool.tile([128, W_ALL], f16, tag="xc")
                for m in range(NMETA):
                    src = x_in[m, lo:hi, :].rearrange("(p t) f -> p (t f)", p=128)
                    nc.sync.dma_start(out=xc[:, m * W_CAT:(m + 1) * W_CAT], in_=src)

                # ---- scores: prod = x*a (2x packed; a_rep seg-broadcast view)
                prod = wpool.tile([128, W_ALL], f16, tag="prod")
                a_bc = a_rep[:, :].rearrange("p (o f) -> p o f", o=1).broadcast_to(
                    [128, SEG, F])
                nc.vector.tensor_tensor(
                    out=prod[:, :].rearrange("p (s f) -> p s f", f=F),
                    in0=xc[:, :].rearrange("p (s f) -> p s f", f=F),
                    in1=a_bc, op=mult)

                # ---- seg-sum: fold tree down to width 8 (@2x packed), then
                # one 1x reduce for the tail (tiny inner runs defeat packing)
                widths = [32, 16, 8]
                cur = prod
                cw = F
                for lvl, hw in enumerate(widths):
                    nxt = wpool.tile([128, SEG * hw], f16, tag=f"fold{lvl}")
                    cin = cur[:, :].rearrange("p (s h) -> p s h", h=cw)
                    nc.vector.tensor_tensor(
                        out=nxt[:, :].rearrange("p (s h) -> p s h", h=hw),
                        in0=cin[:, :, 0:hw], in1=cin[:, :, hw:cw], op=add)
                    cur = nxt
                    cw = hw
                e_raw = spool.tile([128, SEG], f32, tag="e_raw")
                nc.vector.tensor_reduce(
                    out=e_raw[:, :],
                    in_=cur[:, :].rearrange("p (s h) -> p s h", h=8),
                    axis=mybir.AxisListType.X,
                    op=add,
                )

                # ---- softmax pieces: prelu+exp on Scalar, sums/recip on DVE
                u = spool.tile([128, SEG], f32, tag="u")
                nc.scalar.activation(u[:, :], e_raw[:, :], AF.Prelu,
                                     alpha=alpha_c[:, :])
                nc.scalar.activation(u[:, :], u[:, :], AF.Exp)

                s = spool.tile([128, T], f32, tag="s")
                nc.vector.tensor_reduce(
                    out=s[:, :],
                    in_=u[:, :].rearrange("p (m t) -> p t m", m=NMETA),
                    axis=mybir.AxisListType.X,
                    op=add,
                )
                r = spool.tile([128, T], f32, tag="r")
                nc.vector.reciprocal(r[:, :], s[:, :])
                w = spool.tile([128, SEG], f16, tag="w")
                r_bc = r[:, :].rearrange("p (o t) -> p o t", o=1).broadcast_to(
                    [128, NMETA, T])
                nc.vector.tensor_tensor(
                    out=w[:, :].rearrange("p (m t) -> p m t", m=NMETA),
                    in0=u[:, :].rearrange("p (m t) -> p m t", m=NMETA),
                    in1=r_bc, op=mult)

                # ---- weighted sum: y_m = w_m (bcast over f) * x_m
                # m=0: Scalar expands W0, DVE multiplies packed; m=1..3 GpSimd.
                y = wpool.tile([128, W_ALL], f16, tag="y")
                w0x = spool.tile([128, W_CAT], f16, tag="w0x")
                w0_bc = w[:, 0:T].rearrange("p (t o) -> p t o", o=1).broadcast_to(
                    [128, T, F])
                nc.scalar.activation(
                    w0x[:, :].rearrange("p (t f) -> p t f", f=F), w0_bc, AF.Copy)
                nc.vector.tensor_tensor(
                    out=y[:, 0:W_CAT], in0=xc[:, 0:W_CAT], in1=w0x[:, :], op=mult)
                for m in range(1, NMETA):
                    w_bc = w[:, m * T:(m + 1) * T].rearrange(
                        "p (t o) -> p t o", o=1).broadcast_to([128, T, F])
                    xm3 = xc[:, m * W_CAT:(m + 1) * W_CAT].rearrange(
                        "p (t f) -> p t f", f=F)
                    ym3 = y[:, m * W_CAT:(m + 1) * W_CAT].rearrange(
                        "p (t f) -> p t f", f=F)
                    nc.gpsimd.tensor_tensor(out=ym3, in0=xm3, in1=w_bc, op=mult)

                # ---- sum over m on TensorE: psum[:, j] += I.T @ y_m[:, j]
                acc = ppool.tile([128, W_CAT], f32, tag="acc")
                for j in range(W_CAT // 512):
                    js = j * 512
                    for m in range(NMETA):
                        nc.tensor.matmul(
                            acc[:, js:js + 512],
                            ident[:, :],
                            y[:, m * W_CAT + js:m * W_CAT + js + 512],
                            start=(m == 0),
                            stop=(m == NMETA - 1),
                        )

                # ---- drain PSUM -> SBUF (fp16) on Scalar, then store
                ot = wpool.tile([128, W_CAT], f16, tag="ot")
                nc.scalar.copy(ot[:, :], acc[:, :])
                dst = out[lo:hi, :].rearrange("(p t) f -> p (t f)", p=128)
                nc.sync.dma_start(out=dst, in_=ot[:, :])

    nc.compile()
    return nc


def kernel(input, a, _trace=False):
    a = np.asarray(a, dtype=np.float32).reshape(F)

    if "nc" not in _CACHE:
        _CACHE["nc"] = _build_kernel()
    nc = _CACHE["nc"]

    x16 = np.asarray(input).astype(np.float16)
    pad = N_PAD - x16.shape[1]
    if pad:
        x16 = np.concatenate(
            [x16, np.zeros((NMETA, pad, F), np.float16)], axis=1)

    a16 = a.astype(np.float16)
    a_rep = np.tile(a16[None, :], (128, 1))
    ident = np.eye(128, dtype=np.float16)

    in_maps = []
    for c in range(N_CORES):
        sl = x16[:, c * NC_NODES:(c + 1) * NC_NODES, :]
        in_maps.append({
            "input": np.ascontiguousarray(sl),
            "a_rep": a_rep,
            "ident": ident,
        })

    res = bass_utils.run_bass_kernel_spmd(
        nc, in_maps, core_ids=list(range(N_CORES)), trace=_trace
    )
    outs = [res.results[c]["out"] for c in range(N_CORES)]
    full = np.concatenate(outs, axis=0)[:N_FULL].astype(np.float32)
    if _trace:
        return full, res
    return full


# revision 8
# speedup vs baseline: 1.6568x; 1.0986x over previous
"""MetapathAggrLayer Trainium2 kernel — v5 (fp16, fold-tree scores, engine-balanced).

Per node n: e_m = leakyrelu(x[m,n,:].a), w = softmax(e), out = sum_m w_m x[m,n,:].
Data-parallel over N across 8 NeuronCores; nodes-on-partitions layout.

Engine budget per 4096-node batch (measured):
  DVE:    score-mult @2x (4.9us) + fold-tree seg-sum @2x (5.5us) + softmax
          smalls (0.7us) + 1 packed hadamard (1.2us)
  GpSimd: 3 broadcast hadamards (10.8us)
  Scalar: prelu+exp (1us) + W0 expansion (2us) + PSUM drain (2us)
  TensorE: metapath sum via identity-matmul PSUM accumulation (12.6us)
  DMA:    2.5 MB (9us)
"""

import sys

sys.path.insert(0, "/opt/trn_rl_repo")

import numpy as np

import concourse.bacc as bacc
import concourse.mybir as mybir
from concourse import bass_utils
from concourse.tile import TileContext

ALPHA = 0.2
NMETA = 4
F = 64
N_FULL = 1_000_000
N_CORES = 8
T = 32                     # node-chunks per partition per batch
NODES_PER_BATCH = 128 * T  # 4096
BATCHES_PER_CORE = 31
NC_NODES = BATCHES_PER_CORE * NODES_PER_BATCH  # 126_976
N_PAD = N_CORES * NC_NODES                     # 1_015_808
SEG = NMETA * T            # 128 score segments per partition per batch
W_CAT = T * F              # 2048: free width of one metapath tile
W_ALL = NMETA * W_CAT      # 8192: free width of the concatenated x tile

_CACHE = {}


def _build_kernel():
    nc = bacc.Bacc("TRN2", target_bir_lowering=False, debug=False)
    f16 = mybir.dt.float16
    f32 = mybir.dt.float32

    x_in = nc.dram_tensor("input", (NMETA, NC_NODES, F), f16, kind="ExternalInput").ap()
    a_rep_in = nc.dram_tensor("a_rep", (128, F), f16, kind="ExternalInput").ap()
    ident_in = nc.dram_tensor("ident", (128, 128), f16, kind="ExternalInput").ap()
    out = nc.dram_tensor("out", (NC_NODES, F), f16, kind="ExternalOutput").ap()

    mult = mybir.AluOpType.mult
    add = mybir.AluOpType.add
    AF = mybir.ActivationFunctionType

    with TileContext(nc) as tc:
        with tc.tile_pool(name="const", bufs=1) as cpool, \
             tc.tile_pool(name="xbuf", bufs=3) as xpool, \
             tc.tile_pool(name="work", bufs=2) as wpool, \
             tc.tile_pool(name="small", bufs=2) as spool, \
             tc.tile_pool(name="psum", bufs=2, space="PSUM") as ppool:
            a_rep = cpool.tile([128, F], f16)
            ident = cpool.tile([128, 128], f16)
            alpha_c = cpool.tile([128, 1], f32)
            nc.sync.dma_start(out=a_rep[:, :], in_=a_rep_in)
            nc.sync.dma_start(out=ident[:, :], in_=ident_in)
            nc.gpsimd.memset(alpha_c[:, :], ALPHA)

            for i in range(BATCHES_PER_CORE):
                lo = i * NODES_PER_BATCH
                hi = lo + NODES_PER_BATCH

                # ---- load the 4 metapath slices into one wide tile
                xc = xpool.tile([128, W_ALL], f16, tag="xc")
                for m in range(NMETA):
                    src = x_in[m, lo:hi, :].rearrange("(p t) f -> p (t f)", p=128)
                    nc.sync.dma_start(out=xc[:, m * W_CAT:(m + 1) * W_CAT], in_=src)

                # ---- scores: prod = x*a (2x packed; a_rep seg-broadcast view)
                prod = wpool.tile([128, W_ALL], f16, tag="prod")
                a_bc = a_rep[:, :].rearrange("p (o f) -> p o f", o=1).broadcast_to(
                    [128, SEG, F])
                nc.vector.tensor_tensor(
                    out=prod[:, :].rearrange("p (s f) -> p s f", f=F),
                    in0=xc[:, :].rearrange("p (s f) -> p s f", f=F),
                    in1=a_bc, op=mult)

                # ---- seg-sum: fold tree down to width 8 (@2x packed), then
                # one 1x reduce for the tail (tiny inner runs defeat packing)
                widths = [32, 16, 8]
                cur = prod
                cw = F
                for lvl, hw in enumerate(widths):
                    nxt = wpool.tile([128, SEG * hw], f16, tag=f"fold{lvl}")
                    cin = cur[:, :].rearrange("p (s h) -> p s h", h=cw)
                    nc.vector.tensor_tensor(
                        out=nxt[:, :].rearrange("p (s h) -> p s h", h=hw),
                        in0=cin[:, :, 0:hw], in1=cin[:, :, hw:cw], op=add)
                    cur = nxt
                    cw = hw
                e_raw = spool.tile([128, SEG], f32, tag="e_raw")
                nc.vector.tensor_reduce(
                    out=e_raw[:, :],
                    in_=cur[:, :].rearrange("p (s h) -> p s h", h=8),
                    axis=mybir.AxisListType.X,
                    op=add,
                )

                # ---- softmax pieces: prelu+exp on Scalar, sums/recip on DVE
                u = spool.tile([128, SEG], f32, tag="u")
                nc.scalar.activation(u[:, :], e_raw[:, :], AF.Prelu,
                                     alpha=alpha_c[:, :])
                nc.scalar.activation(u[:, :], u[:, :], AF.Exp)

                s = spool.tile([128, T], f32, tag="s")
                nc.vector.tensor_reduce(
                    out=s[:, :],
                    in_=u[:, :].rearrange("p (m t) -> p t m", m=NMETA),
                    axis=mybir.AxisListType.X,
                    op=add,
                )
                r = spool.tile([128, T], f32, tag="r")
                nc.vector.reciprocal(r[:, :], s[:, :])
                w = spool.tile([128, SEG], f16, tag="w")
                r_bc = r[:, :].rearrange("p (o t) -> p o t", o=1).broadcast_to(
                    [128, NMETA, T])
                nc.vector.tensor_tensor(
                    out=w[:, :].rearrange("p (m t) -> p m t", m=NMETA),
                    in0=u[:, :].rearrange("p (m t) -> p m t", m=NMETA),
                    in1=r_bc, op=mult)

                # ---- weighted sum: Scalar expands w_m into y (own SBUF ports,
                # overlaps DVE), then DVE multiplies in-place at 2x packed.
                # GpSimd stays idle: its tensor ops hold the DVE/GpSimd shared
                # SBUF port pair for their whole duration and serialize with
                # every 2-source DVE op.
                y = wpool.tile([128, W_ALL], f16, tag="y")
                for m in range(NMETA):
                    w_bc = w[:, m * T:(m + 1) * T].rearrange(
                        "p (t o) -> p t o", o=1).broadcast_to([128, T, F])
                    ym3 = y[:, m * W_CAT:(m + 1) * W_CAT].rearrange(
                        "p (t f) -> p t f", f=F)
                    nc.scalar.activation(ym3, w_bc, AF.Copy)
                for m in range(NMETA):
                    ys = y[:, m * W_CAT:(m + 1) * W_CAT]
                    nc.vector.tensor_tensor(
                        out=ys, in0=ys, in1=xc[:, m * W_CAT:(m + 1) * W_CAT],
                        op=mult)

                # ---- sum over m on TensorE: psum[:, j] += I.T @ y_m[:, j]
                acc = ppool.tile([128, W_CAT], f32, tag="acc")
                for j in range(W_CAT // 512):
                    js = j * 512
                    for m in range(NMETA):
                        nc.tensor.matmul(
                            acc[:, js:js + 512],
                            ident[:, :],
                            y[:, m * W_CAT + js:m * W_CAT + js + 512],
                            start=(m == 0),
                            stop=(m == NMETA - 1),
                        )

                # ---- drain PSUM -> SBUF (fp16) on Scalar, then store
                ot = wpool.tile([128, W_CAT], f16, tag="ot")
                nc.scalar.copy(ot[:, :], acc[:, :])
                dst = out[lo:hi, :].rearrange("(p t) f -> p (t f)", p=128)
                nc.sync.dma_start(out=dst, in_=ot[:, :])

    nc.compile()
    return nc


def kernel(input, a, _trace=False):
    a = np.asarray(a, dtype=np.float32).reshape(F)

    if "nc" not in _CACHE:
        _CACHE["nc"] = _build_kernel()
    nc = _CACHE["nc"]

    x16 = np.asarray(input).astype(np.float16)
    pad = N_PAD - x16.shape[1]
    if pad:
        x16 = np.concatenate(
            [x16, np.zeros((NMETA, pad, F), np.float16)], axis=1)

    a16 = a.astype(np.float16)
    a_rep = np.tile(a16[None, :], (128, 1))
    ident = np.eye(128, dtype=np.float16)

    in_maps = []
    for c in range(N_CORES):
        sl = x16[:, c * NC_NODES:(c + 1) * NC_NODES, :]
        in_maps.append({
            "input": np.ascontiguousarray(sl),
            "a_rep": a_rep,
            "ident": ident,
        })

    res = bass_utils.run_bass_kernel_spmd(
        nc, in_maps, core_ids=list(range(N_CORES)), trace=_trace
    )
    outs = [res.results[c]["out"] for c in range(N_CORES)]
    full = np.concatenate(outs, axis=0)[:N_FULL].astype(np.float32)
    if _trace:
        return full, res
    return full
